# revision 4
# baseline (speedup 1.0000x reference)
"""MoE (B=8192, D=3072, H=4096, E=8, top-2) on 8 trn2 NeuronCores.

Strategy: expert parallelism. Core c holds expert c's weights (w1[c], w2[c]).
Host computes the gating (replicating the reference's jax ops on the default
backend so routing decisions match the reference bit-for-bit), dispatches each
expert's tokens to its core (transposed, zero-padded to a fixed capacity),
and each core runs the expert FFN:

    yT = w2^T @ relu(w1^T @ xT + b1) + b2        (all matmuls in fp32r)

Device layout: stage 1 produces hT (H on partitions, tokens on the free axis)
so stage 2 can contract over H directly; stage 2 produces yT (D on partitions)
so the b2 bias is a per-partition scalar add. Host applies the top-2 gate
weights and scatter-adds the two expert contributions back to token order.
The load-balancing loss is computed from the same gating ops.
"""
import os
from concurrent.futures import ThreadPoolExecutor

import numpy as np

import concourse.mybir as mybir
import concourse.tile as tile
from concourse import bacc
from concourse.bass_utils import run_bass_kernel_spmd

B, D, H, E, K = 8192, 3072, 4096, 8, 2
P = 128
CAP = 2304                    # per-expert token capacity (max observed load + pad to 128)
BLOCKS = [512, 512, 512, 512, 256]   # token blocks per core (sum == CAP)
KO = D // P                   # 24 k-tiles over D
MO = H // P                   # 32 k-tiles over H
F32 = mybir.dt.float32
F32R = mybir.dt.float32r

_CACHE = {}


def _build_nc():
    """Per-core expert-FFN kernel (SPMD; same program on all 8 cores)."""
    nc = bacc.Bacc("TRN2", target_bir_lowering=False, debug=False)
    xT_d = nc.dram_tensor("xT", [D, CAP], F32R, kind="ExternalInput")
    w1_d = nc.dram_tensor("w1", [D, H], F32R, kind="ExternalInput")
    w2_d = nc.dram_tensor("w2", [H, D], F32R, kind="ExternalInput")
    b1_d = nc.dram_tensor("b1t", [P, MO], F32, kind="ExternalInput")
    b2_d = nc.dram_tensor("b2t", [P, KO], F32, kind="ExternalInput")
    yT_d = nc.dram_tensor("yT", [D, CAP], F32, kind="ExternalOutput")

    w1_r = w1_d.rearrange("(ko ki) h -> ki ko h", ki=P)
    w2_r = w2_d.rearrange("(mo mi) d -> mi mo d", mi=P)
    xT_r = xT_d.rearrange("(ko ki) t -> ki ko t", ki=P)

    with tile.TileContext(nc) as tc:
        with (
            tc.tile_pool(name="const", bufs=1) as const,
            tc.tile_pool(name="xpool", bufs=1) as xpool,
            tc.tile_pool(name="hpool", bufs=MO) as hpool,
            tc.tile_pool(name="w1pool", bufs=8) as w1pool,
            tc.tile_pool(name="w2pool", bufs=8) as w2pool,
            tc.tile_pool(name="opool", bufs=8) as opool,
            tc.tile_pool(name="ps", bufs=8, space="PSUM") as ps,
        ):
            b1sb = const.tile([P, MO], F32)
            nc.sync.dma_start(b1sb[:], b1_d[:])
            b2sb = const.tile([P, KO], F32)
            nc.sync.dma_start(b2sb[:], b2_d[:])

            off = 0
            for tb in BLOCKS:
                xt = xpool.tile([P, KO, tb], F32R, tag="xt")
                nc.sync.dma_start(xt[:], xT_r[:, :, off:off + tb])

                # stage 1: hT[m] = relu(w1^T x + b1), H on partitions
                hT = []
                for mg in range(MO // 4):
                    psums = [ps.tile([P, tb], F32, tag="ps", name=f"ps{_s}") for _s in range(4)]
                    for ko in range(KO):
                        w1t = w1pool.tile([P, 512], F32R, tag="w1t")
                        nc.sync.dma_start(w1t[:], w1_r[:, ko, mg * 512:(mg + 1) * 512])
                        for s in range(4):
                            nc.tensor.matmul(
                                psums[s][:], w1t[:, s * P:(s + 1) * P], xt[:, ko],
                                start=(ko == 0), stop=(ko == KO - 1),
                            )
                    for s in range(4):
                        m = mg * 4 + s
                        ht = hpool.tile([P, tb], F32R, tag="hT")
                        nc.scalar.activation(
                            ht[:], psums[s][:], mybir.ActivationFunctionType.Relu,
                            bias=b1sb[:, m:m + 1],
                        )
                        hT.append(ht)

                # stage 2: yT = w2^T hT + b2, D on partitions
                for dg in range(KO // 4):
                    psums = [ps.tile([P, tb], F32, tag="ps", name=f"ps{_s}") for _s in range(4)]
                    for m in range(MO):
                        w2t = w2pool.tile([P, 512], F32R, tag="w2t")
                        nc.sync.dma_start(w2t[:], w2_r[:, m, dg * 512:(dg + 1) * 512])
                        for s in range(4):
                            nc.tensor.matmul(
                                psums[s][:], w2t[:, s * P:(s + 1) * P], hT[m][:],
                                start=(m == 0), stop=(m == MO - 1),
                            )
                    for s in range(4):
                        dcol = dg * 4 + s
                        ysb = opool.tile([P, tb], F32, tag="ysb")
                        nc.vector.tensor_scalar_add(ysb[:], psums[s][:], b2sb[:, dcol:dcol + 1])
                        nc.sync.dma_start(yT_d[dcol * P:(dcol + 1) * P, off:off + tb], ysb[:])
                off += tb

    nc.compile()
    return nc


def _gating(x, gate_w, gate_b):
    """Replicate the reference's gating math with eager jax ops on the default
    backend (same compiled ops as the reference => same routing decisions)."""
    import jax
    import jax.numpy as jnp

    xj = jnp.asarray(x)
    gate_logits = xj @ jnp.asarray(gate_w) + jnp.asarray(gate_b)
    top_v, top_i = jax.lax.top_k(gate_logits, K)
    top_gates = jax.nn.softmax(top_v, axis=-1)
    gate_probs = jax.nn.softmax(gate_logits, axis=-1)
    mean_probs = jnp.mean(gate_probs, axis=0)
    lb_loss = jnp.mean((mean_probs - 1.0 / E) ** 2) * E
    return (np.asarray(top_i), np.asarray(top_gates), np.asarray(lb_loss))


def _ffn_host(x_rows, w1e, b1e, w2e, b2e):
    h = np.maximum(x_rows @ w1e + b1e, 0.0)
    return h @ w2e + b2e


def kernel(x, gate_w, gate_b, w1, b1, w2, b2, _trace=False):
    x = np.asarray(x)
    top_i, top_g, lb_loss = _gating(x, gate_w, gate_b)
    w1 = np.asarray(w1)
    b1 = np.asarray(b1)
    w2 = np.asarray(w2)
    b2 = np.asarray(b2)

    if "nc" not in _CACHE:
        _CACHE["nc"] = _build_nc()
    nc = _CACHE["nc"]

    rows_e, gates_e = [], []
    for e in range(E):
        sel = top_i == e                       # (B, K) bool
        rows = np.flatnonzero(sel.any(axis=1))
        rows_e.append(rows)
        gates_e.append((top_g * sel).sum(axis=1)[rows].astype(np.float32))

    def prep(e):
        rows = rows_e[e]
        n = min(len(rows), CAP)
        xT = np.zeros((D, CAP), np.float32)
        xT[:, :n] = x[rows[:n]].T
        return {
            "xT": xT,
            "w1": np.ascontiguousarray(w1[e]),
            "w2": np.ascontiguousarray(w2[e]),
            "b1t": np.ascontiguousarray(b1[e].reshape(MO, P).T),
            "b2t": np.ascontiguousarray(b2[e].reshape(KO, P).T),
        }

    with ThreadPoolExecutor(8) as pool:
        in_maps = list(pool.map(prep, range(E)))

    res = run_bass_kernel_spmd(nc, in_maps, core_ids=list(range(E)), trace=_trace)
    if _trace:
        _CACHE["last_results"] = res

    out = np.zeros((B, D), np.float32)
    for e in range(E):
        rows = rows_e[e]
        n = min(len(rows), CAP)
        yT = res.results[e]["yT"]
        out[rows[:n]] += gates_e[e][:n, None] * yT[:, :n].T
        if len(rows) > n:  # capacity overflow: finish the tail on host
            r = rows[n:]
            out[r] += gates_e[e][n:, None] * _ffn_host(
                x[r], w1[e], b1[e], w2[e], b2[e]
            )
    return out, lb_loss


# revision 6
# speedup vs baseline: 1.0185x; 1.0185x over previous
"""MoE (B=8192, D=3072, H=4096, E=8, top-2) on 8 trn2 NeuronCores.

Strategy: expert parallelism. Core c holds expert c's weights (w1[c], w2[c]).
Host computes the gating (replicating the reference's jax ops on the default
backend so routing decisions match the reference bit-for-bit), dispatches each
expert's tokens to its core (transposed, zero-padded to a fixed capacity),
and each core runs the expert FFN:

    yT = w2^T @ relu(w1^T @ xT + b1) + b2        (all matmuls in fp32r)

Device layout: stage 1 produces hT (H on partitions, tokens on the free axis)
so stage 2 can contract over H directly; stage 2 produces yT (D on partitions)
so the b2 bias is a per-partition scalar add. Host applies the top-2 gate
weights and scatter-adds the two expert contributions back to token order.
The load-balancing loss is computed from the same gating ops.
"""
import os
from concurrent.futures import ThreadPoolExecutor

import numpy as np

import concourse.mybir as mybir
import concourse.tile as tile
from concourse import bacc
from concourse.bass_utils import run_bass_kernel_spmd

B, D, H, E, K = 8192, 3072, 4096, 8, 2
P = 128
CAP = 2304                    # per-expert token capacity (max observed load + pad to 128)
BLOCKS = [512, 512, 512, 512, 256]   # token blocks per core (sum == CAP)
KO = D // P                   # 24 k-tiles over D
MO = H // P                   # 32 k-tiles over H
F32 = mybir.dt.float32
F32R = mybir.dt.float32r

_CACHE = {}


def _build_nc():
    """Per-core expert-FFN kernel (SPMD; same program on all 8 cores)."""
    nc = bacc.Bacc("TRN2", target_bir_lowering=False, debug=False)
    xT_d = nc.dram_tensor("xT", [D, CAP], F32R, kind="ExternalInput")
    w1_d = nc.dram_tensor("w1", [D, H], F32R, kind="ExternalInput")
    w2_d = nc.dram_tensor("w2", [H, D], F32R, kind="ExternalInput")
    b1_d = nc.dram_tensor("b1t", [P, MO], F32, kind="ExternalInput")
    b2_d = nc.dram_tensor("b2t", [P, KO], F32, kind="ExternalInput")
    yT_d = nc.dram_tensor("yT", [D, CAP], F32, kind="ExternalOutput")

    w1_r = w1_d.rearrange("(ko ki) h -> ki ko h", ki=P)
    w2_r = w2_d.rearrange("(mo mi) d -> mi mo d", mi=P)
    xT_r = xT_d.rearrange("(ko ki) t -> ki ko t", ki=P)

    with tile.TileContext(nc) as tc:
        with (
            tc.tile_pool(name="const", bufs=1) as const,
            tc.tile_pool(name="xpool", bufs=1) as xpool,
            tc.tile_pool(name="hpool", bufs=MO) as hpool,
            tc.tile_pool(name="w1pool", bufs=6) as w1pool,
            tc.tile_pool(name="w2pool", bufs=24) as w2pool,
            tc.tile_pool(name="opool", bufs=6) as opool,
            tc.tile_pool(name="ps", bufs=8, space="PSUM") as ps,
        ):
            b1sb = const.tile([P, MO], F32)
            nc.sync.dma_start(b1sb[:], b1_d[:])
            b2sb = const.tile([P, KO], F32)
            nc.sync.dma_start(b2sb[:], b2_d[:])

            off = 0
            for tb in BLOCKS:
                xt = xpool.tile([P, KO, tb], F32R, tag="xt")
                for ko in range(KO):  # split across DMA queues
                    nc.sync.dma_start(xt[:, ko], xT_r[:, ko, off:off + tb])

                # stage 1: hT[m] = relu(w1^T x + b1), H on partitions
                hT = []
                for mg in range(MO // 4):
                    psums = [ps.tile([P, tb], F32, tag="ps", name=f"ps{_s}") for _s in range(4)]
                    for ko in range(KO):
                        w1t = w1pool.tile([P, 512], F32R, tag="w1t")
                        nc.sync.dma_start(w1t[:], w1_r[:, ko, mg * 512:(mg + 1) * 512])
                        for s in range(4):
                            nc.tensor.matmul(
                                psums[s][:], w1t[:, s * P:(s + 1) * P], xt[:, ko],
                                start=(ko == 0), stop=(ko == KO - 1),
                            )
                    for s in range(4):
                        m = mg * 4 + s
                        ht = hpool.tile([P, tb], F32R, tag="hT")
                        nc.scalar.activation(
                            ht[:], psums[s][:], mybir.ActivationFunctionType.Relu,
                            bias=b1sb[:, m:m + 1],
                        )
                        hT.append(ht)

                # stage 2: yT = w2^T hT + b2, D on partitions
                for dg in range(KO // 4):
                    psums = [ps.tile([P, tb], F32, tag="ps", name=f"ps{_s}") for _s in range(4)]
                    for m in range(MO):
                        w2t = w2pool.tile([P, 512], F32R, tag="w2t")
                        nc.sync.dma_start(w2t[:], w2_r[:, m, dg * 512:(dg + 1) * 512])
                        for s in range(4):
                            nc.tensor.matmul(
                                psums[s][:], w2t[:, s * P:(s + 1) * P], hT[m][:],
                                start=(m == 0), stop=(m == MO - 1),
                            )
                    for s in range(4):
                        dcol = dg * 4 + s
                        ysb = opool.tile([P, tb], F32, tag="ysb")
                        nc.vector.tensor_scalar_add(ysb[:], psums[s][:], b2sb[:, dcol:dcol + 1])
                        nc.sync.dma_start(yT_d[dcol * P:(dcol + 1) * P, off:off + tb], ysb[:])
                off += tb

    nc.compile()
    return nc


def _gating(x, gate_w, gate_b):
    """Replicate the reference's gating math with eager jax ops on the default
    backend (same compiled ops as the reference => same routing decisions)."""
    import jax
    import jax.numpy as jnp

    xj = jnp.asarray(x)
    gate_logits = xj @ jnp.asarray(gate_w) + jnp.asarray(gate_b)
    top_v, top_i = jax.lax.top_k(gate_logits, K)
    top_gates = jax.nn.softmax(top_v, axis=-1)
    gate_probs = jax.nn.softmax(gate_logits, axis=-1)
    mean_probs = jnp.mean(gate_probs, axis=0)
    lb_loss = jnp.mean((mean_probs - 1.0 / E) ** 2) * E
    return (np.asarray(top_i), np.asarray(top_gates), np.asarray(lb_loss))


def _ffn_host(x_rows, w1e, b1e, w2e, b2e):
    h = np.maximum(x_rows @ w1e + b1e, 0.0)
    return h @ w2e + b2e


def kernel(x, gate_w, gate_b, w1, b1, w2, b2, _trace=False):
    x = np.asarray(x)
    top_i, top_g, lb_loss = _gating(x, gate_w, gate_b)
    w1 = np.asarray(w1)
    b1 = np.asarray(b1)
    w2 = np.asarray(w2)
    b2 = np.asarray(b2)

    if "nc" not in _CACHE:
        _CACHE["nc"] = _build_nc()
    nc = _CACHE["nc"]

    rows_e, gates_e = [], []
    for e in range(E):
        sel = top_i == e                       # (B, K) bool
        rows = np.flatnonzero(sel.any(axis=1))
        rows_e.append(rows)
        gates_e.append((top_g * sel).sum(axis=1)[rows].astype(np.float32))

    def prep(e):
        rows = rows_e[e]
        n = min(len(rows), CAP)
        xT = np.zeros((D, CAP), np.float32)
        xT[:, :n] = x[rows[:n]].T
        return {
            "xT": xT,
            "w1": np.ascontiguousarray(w1[e]),
            "w2": np.ascontiguousarray(w2[e]),
            "b1t": np.ascontiguousarray(b1[e].reshape(MO, P).T),
            "b2t": np.ascontiguousarray(b2[e].reshape(KO, P).T),
        }

    with ThreadPoolExecutor(8) as pool:
        in_maps = list(pool.map(prep, range(E)))

    res = run_bass_kernel_spmd(nc, in_maps, core_ids=list(range(E)), trace=_trace)
    if _trace:
        _CACHE["last_results"] = res

    out = np.zeros((B, D), np.float32)
    for e in range(E):
        rows = rows_e[e]
        n = min(len(rows), CAP)
        yT = res.results[e]["yT"]
        out[rows[:n]] += gates_e[e][:n, None] * yT[:, :n].T
        if len(rows) > n:  # capacity overflow: finish the tail on host
            r = rows[n:]
            out[r] += gates_e[e][n:, None] * _ffn_host(
                x[r], w1[e], b1[e], w2[e], b2[e]
            )
    return out, lb_loss


# revision 7
# speedup vs baseline: 1.0200x; 1.0014x over previous
"""MoE (B=8192, D=3072, H=4096, E=8, top-2) on 8 trn2 NeuronCores.

Strategy: expert parallelism. Core c holds expert c's weights (w1[c], w2[c]).
Host computes the gating (replicating the reference's jax ops on the default
backend so routing decisions match the reference bit-for-bit), dispatches each
expert's tokens to its core (transposed, zero-padded to a fixed capacity),
and each core runs the expert FFN in fp32r:

    yT = w2^T @ relu(w1^T @ xT + b1) + b2

Device schedule: token blocks of 1024 (so each block streams w1/w2 from HBM
only once -> ~200 GB/s steady DMA demand, well under the ~360 GB/s roofline,
keeping the PE at its ~227 ns/matmul fp32r issue rate). Within a block the
H axis is processed in two halves (SBUF can't hold h for 1024 tokens at fp32)
with phases s1(h0) -> s2(h0) -> s1(h1) -> s2(h1); each stage-2 half emits a
partial yT slab and the host sums the two slabs during the gate-weighted
combine. Stage 1 keeps H on partitions (bias+relu fused in the PSUM->SBUF
eviction); stage 2 keeps D on partitions (b2 is a per-partition scalar add,
applied in the h1 slab only).
"""
from concurrent.futures import ThreadPoolExecutor

import numpy as np

import concourse.mybir as mybir
import concourse.tile as tile
from concourse import bacc
from concourse.bass_utils import run_bass_kernel_spmd

B, D, H, E, K = 8192, 3072, 4096, 8, 2
P = 128
CAP = 2304                    # per-expert token capacity (max observed load + pad to 128)
BLOCKS = [1024, 1024, 256]    # token blocks per core (sum == CAP)
KO = D // P                   # 24 k-tiles over D
MO = H // P                   # 32 k-tiles over H
MH = MO // 2                  # m-tiles per H-half
F32 = mybir.dt.float32
F32R = mybir.dt.float32r

_CACHE = {}


def _chunks(tb):
    return [(c, min(512, tb - c)) for c in range(0, tb, 512)]


def _build_nc():
    """Per-core expert-FFN kernel (SPMD; same program on all 8 cores)."""
    nc = bacc.Bacc("TRN2", target_bir_lowering=False, debug=False)
    xT_d = nc.dram_tensor("xT", [D, CAP], F32R, kind="ExternalInput")
    w1_d = nc.dram_tensor("w1", [D, H], F32R, kind="ExternalInput")
    w2_d = nc.dram_tensor("w2", [H, D], F32R, kind="ExternalInput")
    b1_d = nc.dram_tensor("b1t", [P, MO], F32, kind="ExternalInput")
    b2_d = nc.dram_tensor("b2t", [P, KO], F32, kind="ExternalInput")
    yT_d = nc.dram_tensor("yT", [2, D, CAP], F32, kind="ExternalOutput")

    w1_r = w1_d.rearrange("(ko ki) h -> ki ko h", ki=P)
    w2_r = w2_d.rearrange("(mo mi) d -> mi mo d", mi=P)
    xT_r = xT_d.rearrange("(ko ki) t -> ki ko t", ki=P)

    with tile.TileContext(nc) as tc:
        with (
            tc.tile_pool(name="const", bufs=1) as const,
            tc.tile_pool(name="xpool", bufs=1) as xpool,
            tc.tile_pool(name="hpool", bufs=MH) as hpool,
            tc.tile_pool(name="w1pool", bufs=4) as w1pool,
            tc.tile_pool(name="w2pool", bufs=8) as w2pool,
            tc.tile_pool(name="opool", bufs=4) as opool,
            tc.tile_pool(name="ps", bufs=8, space="PSUM") as ps,
        ):
            b1sb = const.tile([P, MO], F32)
            nc.sync.dma_start(b1sb[:], b1_d[:])
            b2sb = const.tile([P, KO], F32)
            nc.sync.dma_start(b2sb[:], b2_d[:])

            off = 0
            for tb in BLOCKS:
                cks = _chunks(tb)
                xt = xpool.tile([P, KO, tb], F32R, tag="xt")
                for ko in range(KO):  # split across DMA queues
                    nc.sync.dma_start(xt[:, ko], xT_r[:, ko, off:off + tb])

                for h in range(2):
                    # stage 1 (half h): hT[m] = relu(w1^T x + b1)
                    hT = []
                    for mg in range(MH // 4):
                        psums = [
                            [ps.tile([P, cw], F32, tag="ps", name=f"ps{_s}_{_c}")
                             for _c, (_, cw) in enumerate(cks)]
                            for _s in range(4)
                        ]
                        col0 = (h * MH + mg * 4) * P
                        for ko in range(KO):
                            w1t = w1pool.tile([P, 512], F32R, tag="w1t")
                            nc.sync.dma_start(w1t[:], w1_r[:, ko, col0:col0 + 512])
                            for s in range(4):
                                for c, (co, cw) in enumerate(cks):
                                    nc.tensor.matmul(
                                        psums[s][c][:], w1t[:, s * P:(s + 1) * P],
                                        xt[:, ko, co:co + cw],
                                        start=(ko == 0), stop=(ko == KO - 1),
                                    )
                        for s in range(4):
                            m = h * MH + mg * 4 + s
                            ht = hpool.tile([P, tb], F32R, tag="hT")
                            for c, (co, cw) in enumerate(cks):
                                nc.scalar.activation(
                                    ht[:, co:co + cw], psums[s][c][:],
                                    mybir.ActivationFunctionType.Relu,
                                    bias=b1sb[:, m:m + 1],
                                )
                            hT.append(ht)

                    # stage 2 (half h): yT[h] partial = w2[half]^T hT (+ b2 in h1)
                    for dg in range(KO // 4):
                        psums = [
                            [ps.tile([P, cw], F32, tag="ps", name=f"ps{_s}_{_c}")
                             for _c, (_, cw) in enumerate(cks)]
                            for _s in range(4)
                        ]
                        for mi in range(MH):
                            m = h * MH + mi
                            w2t = w2pool.tile([P, 512], F32R, tag="w2t")
                            nc.sync.dma_start(w2t[:], w2_r[:, m, dg * 512:(dg + 1) * 512])
                            for s in range(4):
                                for c, (co, cw) in enumerate(cks):
                                    nc.tensor.matmul(
                                        psums[s][c][:], w2t[:, s * P:(s + 1) * P],
                                        hT[mi][:, co:co + cw],
                                        start=(mi == 0), stop=(mi == MH - 1),
                                    )
                        for s in range(4):
                            dcol = dg * 4 + s
                            for c, (co, cw) in enumerate(cks):
                                ysb = opool.tile([P, 512], F32, tag="ysb")
                                if h == 0:
                                    nc.vector.tensor_copy(ysb[:, :cw], psums[s][c][:])
                                else:
                                    nc.vector.tensor_scalar_add(
                                        ysb[:, :cw], psums[s][c][:], b2sb[:, dcol:dcol + 1]
                                    )
                                nc.sync.dma_start(
                                    yT_d[h, dcol * P:(dcol + 1) * P, off + co:off + co + cw],
                                    ysb[:, :cw],
                                )
                off += tb

    nc.compile()
    return nc


def _gating(x, gate_w, gate_b):
    """Replicate the reference's gating math with eager jax ops on the default
    backend (same compiled ops as the reference => same routing decisions)."""
    import jax
    import jax.numpy as jnp

    xj = jnp.asarray(x)
    gate_logits = xj @ jnp.asarray(gate_w) + jnp.asarray(gate_b)
    top_v, top_i = jax.lax.top_k(gate_logits, K)
    top_gates = jax.nn.softmax(top_v, axis=-1)
    gate_probs = jax.nn.softmax(gate_logits, axis=-1)
    mean_probs = jnp.mean(gate_probs, axis=0)
    lb_loss = jnp.mean((mean_probs - 1.0 / E) ** 2) * E
    return (np.asarray(top_i), np.asarray(top_gates), np.asarray(lb_loss))


def _ffn_host(x_rows, w1e, b1e, w2e, b2e):
    h = np.maximum(x_rows @ w1e + b1e, 0.0)
    return h @ w2e + b2e


def kernel(x, gate_w, gate_b, w1, b1, w2, b2, _trace=False):
    x = np.asarray(x)
    top_i, top_g, lb_loss = _gating(x, gate_w, gate_b)
    w1 = np.asarray(w1)
    b1 = np.asarray(b1)
    w2 = np.asarray(w2)
    b2 = np.asarray(b2)

    if "nc" not in _CACHE:
        _CACHE["nc"] = _build_nc()
    nc = _CACHE["nc"]

    rows_e, gates_e = [], []
    for e in range(E):
        sel = top_i == e                       # (B, K) bool
        rows = np.flatnonzero(sel.any(axis=1))
        rows_e.append(rows)
        gates_e.append((top_g * sel).sum(axis=1)[rows].astype(np.float32))

    def prep(e):
        rows = rows_e[e]
        n = min(len(rows), CAP)
        xT = np.zeros((D, CAP), np.float32)
        xT[:, :n] = x[rows[:n]].T
        return {
            "xT": xT,
            "w1": np.ascontiguousarray(w1[e]),
            "w2": np.ascontiguousarray(w2[e]),
            "b1t": np.ascontiguousarray(b1[e].reshape(MO, P).T),
            "b2t": np.ascontiguousarray(b2[e].reshape(KO, P).T),
        }

    with ThreadPoolExecutor(8) as pool:
        in_maps = list(pool.map(prep, range(E)))

    res = run_bass_kernel_spmd(nc, in_maps, core_ids=list(range(E)), trace=_trace)
    if _trace:
        _CACHE["last_results"] = res

    out = np.zeros((B, D), np.float32)
    for e in range(E):
        rows = rows_e[e]
        n = min(len(rows), CAP)
        yT = res.results[e]["yT"]
        g = gates_e[e][:n, None]
        out[rows[:n]] += g * yT[0, :, :n].T
        out[rows[:n]] += g * yT[1, :, :n].T
        if len(rows) > n:  # capacity overflow: finish the tail on host
            r = rows[n:]
            out[r] += gates_e[e][n:, None] * _ffn_host(
                x[r], w1[e], b1[e], w2[e], b2[e]
            )
    return out, lb_loss
